# revision 1
# baseline (speedup 1.0000x reference)
"""LoRALinear fused kernel for 8 trn2 NeuronCores.

y = x @ (base + 2*(B@A))^T + bias,  x:[2,2048,4096], base:[4096,4096],
A:[8,4096], B:[4096,8], bias:[4096] -> y:[2,2048,4096], all fp32.

Sharding: 4 token-shards x 2 dout-shards. Per core:
  y_c[1024, 2048] = x_c[1024,4096] @ W_c[2048,4096]^T + bias_c
decomposed (exact in fp32 up to rounding order) as
  y_c = x_c@base_c^T + [x_c@A^T | 1] @ [2*B_c^T ; bias_c].

Compute runs single-pass float32r (tf32-grade, 1 cycle/row): operands are
rounded to f32r on the host (equivalent to the HW cast path) so every
load is a plain HWDGE DMA. Per core: x^T stays SBUF-resident (16.8MB),
base^T streams once (33.6MB). PSUM: 8 banks = 7 token-tile accumulators +
1 bank timeshared between PT=(A@x^T) and the deferred 8th token tile.
Host does layout/rounding only; all FLOPs are on device.
"""
import sys

sys.path.insert(0, "/opt/trn_rl_repo")

import numpy as np

T_SH, O_SH = 4, 2          # token shards x dout shards
T, D, O = 4096, 4096, 4096  # flattened tokens, d_in, d_out
TC, OC = T // T_SH, O // O_SH    # 1024, 2048 per core
KC = D // 128              # 32 contraction chunks
NB = OC // 512             # 4 o-blocks of 512 per core
TT = TC // 128             # 8 token tiles per core
WG = 4                     # base^T chunks per streaming DMA

_cache = {}


def _round_f32r(a, bits=11):
    """Round fp32 mantissa to `bits` bits, round-to-nearest-even."""
    drop = 23 - bits
    u = np.ascontiguousarray(a, dtype=np.float32).view(np.uint32)
    half = np.uint32((1 << drop) // 2 - 1)
    r = (u + half + ((u >> drop) & 1)) & np.uint32((0xFFFFFFFF >> drop) << drop)
    return r.view(np.float32)


def _build():
    import concourse.mybir as mybir
    import concourse.tile as tile
    from concourse import bacc

    f32 = mybir.dt.float32
    f32r = mybir.dt.float32r

    nc = bacc.Bacc("TRN2", target_bir_lowering=False, debug=False,
                   num_devices=8)

    xt_d = nc.dram_tensor("xt", [D, TC], f32r, kind="ExternalInput").ap()
    wt_d = nc.dram_tensor("wt", [D, OC], f32r, kind="ExternalInput").ap()
    at_d = nc.dram_tensor("at", [128, KC, 8], f32r, kind="ExternalInput").ap()
    # rows 0-7: 2*B^T, row 8: bias  (K=9 close matmul adds lora + bias)
    bb_d = nc.dram_tensor("bb", [9, OC], f32r, kind="ExternalInput").ap()
    ones_d = nc.dram_tensor("ones", [1, TC], f32r, kind="ExternalInput").ap()
    y_d = nc.dram_tensor("y", [TC, OC], f32, kind="ExternalOutput").ap()

    with tile.TileContext(nc) as tc:
        with (
            tc.tile_pool(name="res", bufs=1) as res,
            tc.tile_pool(name="wst", bufs=4) as wst,
            tc.tile_pool(name="evac", bufs=3) as evac,
            tc.tile_pool(name="psum", bufs=1, space="PSUM") as psum,
        ):
            # small residents first (scalar = ACT HWDGE ring)
            at = res.tile([128, KC, 8], f32r)
            nc.scalar.dma_start(at[:], at_d[:])
            bb = res.tile([9, OC], f32r)
            nc.scalar.dma_start(bb[:], bb_d[:])
            # ptw rows 0-7: PT = A@x^T (device-computed), row 8: ones
            ptw = res.tile([9, TC], f32r)
            nc.scalar.dma_start(ptw[8:9, :], ones_d[:])
            xt = res.tile([128, KC, TC], f32r)
            xt_src = xt_d.rearrange("(c p) t -> c p t", p=128)
            # split chunk 0 so the first matmuls' data lands fast
            nc.scalar.dma_start(xt[:, 0, 0:256], xt_src[0][:, 0:256])
            nc.scalar.dma_start(xt[:, 0, 256:TC], xt_src[0][:, 256:TC])
            for k in range(1, KC):
                nc.scalar.dma_start(xt[:, k, :], xt_src[k])

            wt_src = wt_d.rearrange("(c p) o -> p c o", p=128)

            def close_and_evac(acc, t, osl, split_out=False):
                nc.tensor.matmul(acc[:], ptw[:, 128 * t:128 * (t + 1)],
                                 bb[:, osl], start=False, stop=True)
                ev = evac.tile([128, 512], f32, name=f"ev{t}", tag="ev")
                nc.vector.tensor_copy(ev[:], acc[:])
                tsl = slice(128 * t, 128 * (t + 1))
                if split_out:
                    # drain the final tiles on both rings to shorten the tail
                    h = slice(osl.start, osl.start + 256)
                    h2 = slice(osl.start + 256, osl.stop)
                    nc.scalar.dma_start(y_d[tsl, h], ev[:, 0:256])
                    nc.sync.dma_start(y_d[tsl, h2], ev[:, 256:512])
                else:
                    nc.scalar.dma_start(y_d[tsl, osl], ev[:])

            def o_block(ob, t_list, with_pt):
                osl = slice(512 * ob, 512 * (ob + 1))
                accs = {
                    t: psum.tile([128, 512], f32, name=f"acc{t}_{ob}",
                                 tag=f"acc{t}")
                    for t in t_list
                }
                if with_pt:
                    # both PT halves run concurrently, in the banks that t6/t7
                    # of this o-block would have used (their slices deferred)
                    ptp = psum.tile([8, 512], f32, name="ptp0", tag="acc6")
                    ptq = psum.tile([8, 512], f32, name="ptp1", tag="acc7")
                # first 4 chunks ride small tiles on their own tag so the
                # NEXT o-block's head data prefetches early (slots free
                # early in the previous block -> PE never idles >3.4us at
                # block boundaries, avoiding HAM re-throttle)
                groups = []
                for g, (c0, ng) in enumerate(
                        [(0, 2), (2, 2)] +
                        [(4 + WG * i, WG) for i in range((KC - 4) // WG)]):
                    wtile = wst.tile([128, ng, 512], f32r,
                                     name=f"wt{ob}_{g}",
                                     tag=("wt0" if ng == 2 else "wt"),
                                     bufs=(2 if ng == 2 else None))
                    if ob == 0 and with_pt and g == 0:
                        # split the very first weight tile for a fast start
                        for j in range(ng):
                            nc.sync.dma_start(
                                wtile[:, j, :], wt_src[:, c0 + j, osl])
                    else:
                        nc.sync.dma_start(
                            wtile[:], wt_src[:, c0:c0 + ng, osl])
                    groups.append((c0, ng, wtile))
                for c0, ng, wtile in groups:
                    for j in range(ng):
                        k = c0 + j
                        if with_pt:
                            nc.tensor.matmul(ptp[:], at[:, k, :],
                                             xt[:, k, 0:512],
                                             start=(k == 0), stop=(k == KC - 1))
                            nc.tensor.matmul(ptq[:], at[:, k, :],
                                             xt[:, k, 512:1024],
                                             start=(k == 0), stop=(k == KC - 1))
                        for t in t_list:
                            nc.tensor.matmul(
                                accs[t][:],
                                xt[:, k, 128 * t:128 * (t + 1)],
                                wtile[:, j, :],
                                start=(k == 0), stop=False)
                if with_pt:
                    nc.vector.tensor_copy(ptw[0:8, 0:512], ptp[:])
                    nc.vector.tensor_copy(ptw[0:8, 512:1024], ptq[:])
                for t in t_list:
                    close_and_evac(accs[t], t, osl, split_out=False)

            o_block(0, list(range(6)), with_pt=True)
            o_block(0, [6, 7], with_pt=False)  # deferred t6/t7 of o-block 0
            for ob in range(1, NB):
                o_block(ob, list(range(TT)), with_pt=False)

    nc.compile()
    return nc


def _get_nc():
    if "nc" not in _cache:
        _cache["nc"] = _build()
    return _cache["nc"]


def kernel(x, base_weight, lora_A, lora_B, bias, _trace=False, _trace_kwargs=None):
    from concourse.bass_utils import run_bass_kernel_spmd

    nc = _get_nc()

    x_flat = np.ascontiguousarray(x, dtype=np.float32).reshape(T, D)
    xT = x_flat.T
    wT = base_weight.T
    at = _round_f32r(np.ascontiguousarray(
        lora_A.T, dtype=np.float32).reshape(KC, 128, 8).transpose(1, 0, 2))
    ones = np.ones((1, TC), dtype=np.float32)

    xt_shards = [_round_f32r(xT[:, TC * i:TC * (i + 1)]) for i in range(T_SH)]
    wt_shards = [_round_f32r(wT[:, OC * i:OC * (i + 1)]) for i in range(O_SH)]
    bb_shards = [
        _round_f32r(np.vstack([2.0 * lora_B[OC * i:OC * (i + 1), :].T,
                               bias[None, OC * i:OC * (i + 1)]]))
        for i in range(O_SH)
    ]

    in_maps = []
    for c in range(8):
        ti, oi = c % T_SH, c // T_SH
        in_maps.append({
            "xt": xt_shards[ti],
            "wt": wt_shards[oi],
            "at": at,
            "bb": bb_shards[oi],
            "ones": ones,
        })

    res = run_bass_kernel_spmd(nc, in_maps, list(range(8)),
                               trace=_trace, **(_trace_kwargs or {}))

    y = np.empty((T, O), dtype=np.float32)
    for c in range(8):
        ti, oi = c % T_SH, c // T_SH
        y[TC * ti:TC * (ti + 1), OC * oi:OC * (oi + 1)] = res.results[c]["y"]
    out = y.reshape(x.shape[0], x.shape[1], O)
    if _trace:
        return out, res
    return out



# revision 2
# speedup vs baseline: 1.2298x; 1.2298x over previous
"""LoRALinear fused kernel for 8 trn2 NeuronCores — v2 (fp16 operands).

y = x @ (base + 2*(B@A))^T + bias,  x:[2,2048,4096], base:[4096,4096],
A:[8,4096], B:[4096,8], bias:[4096] -> y:[2,2048,4096], all fp32.

Sharding: 4 token-shards x 2 dout-shards. Per core:
  y_c[1024, 2048] = x_c[1024,4096] @ W_c[2048,4096]^T + bias_c
decomposed as
  y_c = x_c@base_c^T + [x_c@A^T | 1] @ [2*B_c^T ; bias_c].

v2: all matmul operands are fp16 (PSUM accumulation stays fp32). For
f32r every InstMatmult self-loads its 128x128 stationary (ldw-opt is
broken for f32r) and the 4-byte LDWEIGHTS (224ns) + 54ns handoff
dominates the 213ns moving stream -> 280ns/MM cadence. fp16 halves the
LDW bytes -> cadence ~217ns/MM, and halves x/W HBM traffic. Mixed
dtype is not an option: the walrus verifier requires operand dtypes to
match when either is fp32/f32r. fp16 (10-bit mantissa) over bf16
(8-bit) costs nothing and keeps absmax rel err ~5e-4 (gate 2e-2).
"""
import sys

sys.path.insert(0, "/opt/trn_rl_repo")

import numpy as np

T_SH, O_SH = 4, 2          # token shards x dout shards
T, D, O = 4096, 4096, 4096  # flattened tokens, d_in, d_out
TC, OC = T // T_SH, O // O_SH    # 1024, 2048 per core
KC = D // 128              # 32 contraction chunks
NB = OC // 512             # 4 o-blocks of 512 per core
TT = TC // 128             # 8 token tiles per core
WG = 4                     # base^T chunks per streaming DMA

_cache = {}


def _build():
    import concourse.mybir as mybir
    import concourse.tile as tile
    from concourse import bacc

    f32 = mybir.dt.float32
    fp16 = mybir.dt.float16

    nc = bacc.Bacc("TRN2", target_bir_lowering=False, debug=False,
                   num_devices=8)

    xt_d = nc.dram_tensor("xt", [D, TC], fp16, kind="ExternalInput").ap()
    wt_d = nc.dram_tensor("wt", [D, OC], fp16, kind="ExternalInput").ap()
    at_d = nc.dram_tensor("at", [128, KC, 8], fp16, kind="ExternalInput").ap()
    # rows 0-7: 2*B^T, row 8: bias  (K=9 close matmul adds lora + bias)
    bb_d = nc.dram_tensor("bb", [9, OC], fp16, kind="ExternalInput").ap()
    ones_d = nc.dram_tensor("ones", [1, TC], fp16, kind="ExternalInput").ap()
    y_d = nc.dram_tensor("y", [TC, OC], f32, kind="ExternalOutput").ap()

    with tile.TileContext(nc) as tc:
        with (
            tc.tile_pool(name="res", bufs=1) as res,
            tc.tile_pool(name="wst", bufs=4) as wst,
            tc.tile_pool(name="evac", bufs=3) as evac,
            tc.tile_pool(name="psum", bufs=1, space="PSUM") as psum,
        ):
            # xt chunk 0 first so the first matmuls' data lands fast
            # (scalar = ACT HWDGE ring)
            xt = res.tile([128, KC, TC], fp16)
            xt_src = xt_d.rearrange("(c p) t -> c p t", p=128)
            nc.scalar.dma_start(xt[:, 0, 0:256], xt_src[0][:, 0:256])
            nc.scalar.dma_start(xt[:, 0, 256:TC], xt_src[0][:, 256:TC])
            # small residents
            at = res.tile([128, KC, 8], fp16)
            nc.scalar.dma_start(at[:], at_d[:])
            bb = res.tile([9, OC], fp16)
            nc.scalar.dma_start(bb[:], bb_d[:])
            # ptw rows 0-7: PT = A@x^T (device-computed), row 8: ones
            ptw = res.tile([9, TC], fp16)
            nc.scalar.dma_start(ptw[8:9, :], ones_d[:])
            for k in range(1, KC):
                nc.scalar.dma_start(xt[:, k, :], xt_src[k])

            wt_src = wt_d.rearrange("(c p) o -> p c o", p=128)

            def close_and_evac(acc, t, osl, split_out=False):
                nc.tensor.matmul(acc[:], ptw[:, 128 * t:128 * (t + 1)],
                                 bb[:, osl], start=False, stop=True)
                ev = evac.tile([128, 512], f32, name=f"ev{t}", tag="ev")
                nc.vector.tensor_copy(ev[:], acc[:])
                tsl = slice(128 * t, 128 * (t + 1))
                if split_out:
                    # drain the final tiles on both rings to shorten the tail
                    h = slice(osl.start, osl.start + 256)
                    h2 = slice(osl.start + 256, osl.stop)
                    nc.scalar.dma_start(y_d[tsl, h], ev[:, 0:256])
                    nc.sync.dma_start(y_d[tsl, h2], ev[:, 256:512])
                else:
                    nc.scalar.dma_start(y_d[tsl, osl], ev[:])

            def o_block(ob, t_list, with_pt, last=False):
                osl = slice(512 * ob, 512 * (ob + 1))
                accs = {
                    t: psum.tile([128, 512], f32, name=f"acc{t}_{ob}",
                                 tag=f"acc{t}")
                    for t in t_list
                }
                if with_pt:
                    # both PT halves run concurrently, in the banks that t6/t7
                    # of this o-block would have used (their slices deferred)
                    ptp = psum.tile([8, 512], f32, name="ptp0", tag="acc6")
                    ptq = psum.tile([8, 512], f32, name="ptp1", tag="acc7")
                # first 4 chunks ride small tiles on their own tag so the
                # NEXT o-block's head data prefetches early (slots free
                # early in the previous block -> PE never idles >3.4us at
                # block boundaries, avoiding HAM re-throttle)
                groups = []
                for g, (c0, ng) in enumerate(
                        [(0, 2), (2, 2)] +
                        [(4 + WG * i, WG) for i in range((KC - 4) // WG)]):
                    wtile = wst.tile([128, ng, 512], fp16,
                                     name=f"wt{ob}_{g}",
                                     tag=("wt0" if ng == 2 else "wt"),
                                     bufs=(2 if ng == 2 else None))
                    if ob == 0 and with_pt and g == 0:
                        # split the very first weight tile for a fast start
                        for j in range(ng):
                            nc.sync.dma_start(
                                wtile[:, j, :], wt_src[:, c0 + j, osl])
                    else:
                        nc.sync.dma_start(
                            wtile[:], wt_src[:, c0:c0 + ng, osl])
                    groups.append((c0, ng, wtile))
                for c0, ng, wtile in groups:
                    for j in range(ng):
                        k = c0 + j
                        if with_pt:
                            nc.tensor.matmul(ptp[:], at[:, k, :],
                                             xt[:, k, 0:512],
                                             start=(k == 0), stop=(k == KC - 1))
                            nc.tensor.matmul(ptq[:], at[:, k, :],
                                             xt[:, k, 512:1024],
                                             start=(k == 0), stop=(k == KC - 1))
                        for t in t_list:
                            nc.tensor.matmul(
                                accs[t][:],
                                xt[:, k, 128 * t:128 * (t + 1)],
                                wtile[:, j, :],
                                start=(k == 0), stop=False)
                if with_pt:
                    nc.vector.tensor_copy(ptw[0:8, 0:512], ptp[:])
                    nc.vector.tensor_copy(ptw[0:8, 512:1024], ptq[:])
                for i, t in enumerate(t_list):
                    close_and_evac(accs[t], t, osl,
                                   split_out=(last and i >= len(t_list) - 2))

            o_block(0, list(range(6)), with_pt=True)
            o_block(0, [6, 7], with_pt=False)  # deferred t6/t7 of o-block 0
            for ob in range(1, NB):
                o_block(ob, list(range(TT)), with_pt=False,
                        last=(ob == NB - 1))

    nc.compile()
    return nc


def _get_nc():
    if "nc" not in _cache:
        _cache["nc"] = _build()
    return _cache["nc"]


def kernel(x, base_weight, lora_A, lora_B, bias, _trace=False, _trace_kwargs=None):
    from concourse.bass_utils import run_bass_kernel_spmd

    nc = _get_nc()

    x_flat = np.ascontiguousarray(x, dtype=np.float32).reshape(T, D)
    xT = x_flat.T
    wT = base_weight.T
    at = np.ascontiguousarray(
        lora_A.T, dtype=np.float32).reshape(KC, 128, 8).transpose(1, 0, 2)
    at = at.astype(np.float16)
    ones = np.ones((1, TC), dtype=np.float16)

    xt_shards = [np.ascontiguousarray(
        xT[:, TC * i:TC * (i + 1)]).astype(np.float16) for i in range(T_SH)]
    wt_shards = [np.ascontiguousarray(
        wT[:, OC * i:OC * (i + 1)]).astype(np.float16) for i in range(O_SH)]
    bb_shards = [
        np.vstack([2.0 * lora_B[OC * i:OC * (i + 1), :].T,
                   bias[None, OC * i:OC * (i + 1)]]).astype(np.float16)
        for i in range(O_SH)
    ]

    in_maps = []
    for c in range(8):
        ti, oi = c % T_SH, c // T_SH
        in_maps.append({
            "xt": xt_shards[ti],
            "wt": wt_shards[oi],
            "at": at,
            "bb": bb_shards[oi],
            "ones": ones,
        })

    res = run_bass_kernel_spmd(nc, in_maps, list(range(8)),
                               trace=_trace, **(_trace_kwargs or {}))

    y = np.empty((T, O), dtype=np.float32)
    for c in range(8):
        ti, oi = c % T_SH, c // T_SH
        y[TC * ti:TC * (ti + 1), OC * oi:OC * (oi + 1)] = res.results[c]["y"]
    out = y.reshape(x.shape[0], x.shape[1], O)
    if _trace:
        return out, res
    return out


# revision 3
# speedup vs baseline: 1.3592x; 1.1052x over previous
"""LoRALinear fused kernel for 8 trn2 NeuronCores — v3.

y = x @ (base + 2*(B@A))^T + bias,  x:[2,2048,4096], base:[4096,4096],
A:[8,4096], B:[4096,8], bias:[4096] -> y:[2,2048,4096], all fp32.

Sharding: 8-way token-parallel (data-parallel, replicated weights).
Per core: y_c[512, 4096] = x_c[512,4096] @ W[4096,4096]^T + bias,
decomposed as
  y_c = x_c@base^T + [x_c@A^T | 1] @ [2*B^T ; bias].

All matmul operands are fp16 (PSUM accumulation fp32). For f32r every
InstMatmult self-loads its stationary and the 4-byte LDWEIGHTS (224ns)
+ 54ns handoff beats the 213ns moving stream -> 280ns/MM. fp16 halves
LDW bytes and enables compiler-automatic FWL -> ~216ns/MM (the N=512
stream floor). Mixed dtype is illegal (walrus verifier). fp16 10-bit
mantissa keeps absmax rel err ~2e-4 (gate 2e-2).

Token-parallel (vs 4x2 token x dout) halves the per-core PT = A@x_c^T
overhead (PT moving-cycles scale with tokens/core): one [8,512] PSUM
bank, one PT matmul per k-chunk. Structure per core: 8 o-blocks of 512
douts; per block 4 token-tile accumulators (PSUM tags acc0-3 x bufs=2,
PT rides acc3's second buf). The last o-block runs as two 2-token
passes so the final drain is 2 tiles, not 8; y-DMAs alternate
scalar/sync rings so evac never throttles on one ring.
"""
import sys

sys.path.insert(0, "/opt/trn_rl_repo")

import numpy as np

T_SH = 8                    # token shards (pure data-parallel)
T, D, O = 4096, 4096, 4096  # flattened tokens, d_in, d_out
TC, OC = T // T_SH, O       # 512 tokens per core, full 4096 douts
KC = D // 128               # 32 contraction chunks
NB = OC // 512              # 8 o-blocks of 512
TT = TC // 128              # 4 token tiles per core
WG = 4                      # base^T chunks per streaming DMA

_cache = {}


def _build():
    import concourse.mybir as mybir
    import concourse.tile as tile
    from concourse import bacc

    f32 = mybir.dt.float32
    fp16 = mybir.dt.float16

    nc = bacc.Bacc("TRN2", target_bir_lowering=False, debug=False,
                   num_devices=8)

    xt_d = nc.dram_tensor("xt", [D, TC], fp16, kind="ExternalInput").ap()
    wt_d = nc.dram_tensor("wt", [D, OC], fp16, kind="ExternalInput").ap()
    at_d = nc.dram_tensor("at", [128, KC, 8], fp16, kind="ExternalInput").ap()
    # rows 0-7: 2*B^T, row 8: bias  (K=9 close matmul adds lora + bias)
    bb_d = nc.dram_tensor("bb", [9, OC], fp16, kind="ExternalInput").ap()
    ones_d = nc.dram_tensor("ones", [1, TC], fp16, kind="ExternalInput").ap()
    y_d = nc.dram_tensor("y", [TC, OC], f32, kind="ExternalOutput").ap()

    with tile.TileContext(nc) as tc:
        with (
            tc.tile_pool(name="res", bufs=1) as res,
            tc.tile_pool(name="wst", bufs=8) as wst,
            tc.tile_pool(name="evac", bufs=6) as evac,
            tc.tile_pool(name="psum", bufs=1, space="PSUM") as psum,
        ):
            # xt chunk 0 first so the first matmuls' data lands fast
            # (scalar = ACT HWDGE ring), then at for the PT matmuls
            xt = res.tile([128, KC, TC], fp16)
            xt_src = xt_d.rearrange("(c p) t -> c p t", p=128)
            nc.scalar.dma_start(xt[:, 0, :], xt_src[0])
            at = res.tile([128, KC, 8], fp16)
            nc.scalar.dma_start(at[:], at_d[:])
            for k in range(1, KC):
                nc.scalar.dma_start(xt[:, k, :], xt_src[k])
            # residents not needed until the first closes (~45us in)
            bb = res.tile([9, OC], fp16)
            nc.scalar.dma_start(bb[:], bb_d[:])
            # ptw rows 0-7: PT = A@x^T (device-computed), row 8: ones
            ptw = res.tile([9, TC], fp16)
            nc.scalar.dma_start(ptw[8:9, :], ones_d[:])

            wt_src = wt_d.rearrange("(c p) o -> p c o", p=128)
            ev_ring = [0]

            def close_and_evac(acc, t, osl):
                nc.tensor.matmul(acc[:], ptw[:, 128 * t:128 * (t + 1)],
                                 bb[:, osl], start=False, stop=True)
                ev = evac.tile([128, 512], f32, name=f"ev{t}", tag="ev")
                nc.vector.tensor_copy(ev[:], acc[:])
                tsl = slice(128 * t, 128 * (t + 1))
                ring = nc.scalar if ev_ring[0] % 2 == 0 else nc.sync
                ev_ring[0] += 1
                ring.dma_start(y_d[tsl, osl], ev[:])

            def o_block(ob, t_list, with_pt=False):
                osl = slice(512 * ob, 512 * (ob + 1))
                accs = {
                    t: psum.tile([128, 512], f32, name=f"acc{t}_{ob}",
                                 tag=f"acc{t}", bufs=2)
                    for t in t_list
                }
                # PT rides the second buf of acc3's tag (ob0 only): 4 accs +
                # PT = 5 banks live; later blocks rotate through the 2 bufs
                if with_pt:
                    pt = psum.tile([8, TC], f32, name="pt", tag="acc3",
                                   bufs=2)
                # first 4 chunks ride small tiles on their own tag so the
                # NEXT o-block's head data prefetches early
                groups = []
                for g, (c0, ng) in enumerate(
                        [(0, 2), (2, 2)] +
                        [(4 + WG * i, WG) for i in range((KC - 4) // WG)]):
                    wtile = wst.tile([128, ng, 512], fp16,
                                     name=f"wt{ob}_{g}",
                                     tag=("wt0" if ng == 2 else "wt"),
                                     bufs=(3 if ng == 2 else None))
                    if ob == 0 and with_pt and g == 0:
                        # split the very first weight tile for a fast start
                        for j in range(ng):
                            nc.sync.dma_start(
                                wtile[:, j, :], wt_src[:, c0 + j, osl])
                    else:
                        nc.sync.dma_start(
                            wtile[:], wt_src[:, c0:c0 + ng, osl])
                    groups.append((c0, ng, wtile))
                for c0, ng, wtile in groups:
                    for j in range(ng):
                        k = c0 + j
                        if with_pt:
                            nc.tensor.matmul(pt[:], at[:, k, :], xt[:, k, :],
                                             start=(k == 0), stop=(k == KC - 1))
                        for t in t_list:
                            nc.tensor.matmul(
                                accs[t][:],
                                xt[:, k, 128 * t:128 * (t + 1)],
                                wtile[:, j, :],
                                start=(k == 0), stop=False)
                if with_pt:
                    nc.vector.tensor_copy(ptw[0:8, :], pt[:])
                for t in t_list:
                    close_and_evac(accs[t], t, osl)

            o_block(0, list(range(TT)), with_pt=True)
            for ob in range(1, NB - 1):
                o_block(ob, list(range(TT)))
            # last o-block as two 2-token passes: final drain is 2 tiles
            o_block(NB - 1, [0, 1])
            o_block(NB - 1, [2, 3])

    nc.compile()
    return nc


def _get_nc():
    if "nc" not in _cache:
        _cache["nc"] = _build()
    return _cache["nc"]


def kernel(x, base_weight, lora_A, lora_B, bias, _trace=False, _trace_kwargs=None):
    from concourse.bass_utils import run_bass_kernel_spmd

    nc = _get_nc()

    x_flat = np.ascontiguousarray(x, dtype=np.float32).reshape(T, D)
    xT = x_flat.T
    wt = np.ascontiguousarray(base_weight.T).astype(np.float16)
    at = np.ascontiguousarray(
        lora_A.T, dtype=np.float32).reshape(KC, 128, 8).transpose(1, 0, 2)
    at = at.astype(np.float16)
    bb = np.vstack([2.0 * lora_B.T, bias[None, :]]).astype(np.float16)
    ones = np.ones((1, TC), dtype=np.float16)

    xt_shards = [np.ascontiguousarray(
        xT[:, TC * i:TC * (i + 1)]).astype(np.float16) for i in range(T_SH)]

    in_maps = []
    for c in range(8):
        in_maps.append({
            "xt": xt_shards[c],
            "wt": wt,
            "at": at,
            "bb": bb,
            "ones": ones,
        })

    res = run_bass_kernel_spmd(nc, in_maps, list(range(8)),
                               trace=_trace, **(_trace_kwargs or {}))

    y = np.empty((T, O), dtype=np.float32)
    for c in range(8):
        y[TC * c:TC * (c + 1), :] = res.results[c]["y"]
    out = y.reshape(x.shape[0], x.shape[1], O)
    if _trace:
        return out, res
    return out
